# revision 3
# baseline (speedup 1.0000x reference)
"""Row-scale kernel: C = diag(A) @ B  (scale row i of B by A[i]).

Full shapes: A [16384] f32, B [16384, 4096] f32 -> C [16384, 4096] f32.
Sharding: pure data parallel over rows, 2048 rows per core on 8 cores.

Per-core layout: rows are interleaved over partitions, row r = p*T + t
(p = partition 0..127, t = tile 0..15).  That makes the per-tile scale
vector a_sb[:, t] a plain column of an A tile loaded with ONE contiguous
8 KiB DMA, and each B tile a clean 2D pattern (16 KiB contiguous per
partition, 256 KiB partition stride).

Raw Bass (no Tile framework) with an explicit software pipeline:
  SP sequencer   : B-tile loads  (HWDGE qSP ring)
  DVE            : per-partition scale multiply (in place)
  ACT sequencer  : C-tile stores (HWDGE qAct ring)
Per-buffer-slot semaphores; every instruction carries at most one
embedded wait (standalone sequencer waits otherwise) — the walrus
codegen rejects multi-wait TensorScalar instructions.
"""

import os

import numpy as np

import concourse.bass as bass
import concourse.mybir as mybir
from concourse.bass_utils import run_bass_kernel_spmd

N = 16384
M = 4096
N_CORES = 8
ROWS = N // N_CORES  # 2048 rows per core
P = 128              # SBUF partitions
T = ROWS // P        # 16 row-tiles per core
K = 8                # pipeline buffer slots (K * 16KiB = 128KiB per partition)

_nc_cache = {}
last_exec_time_ns = None


def _build_nc(reps=1):
    """reps>1 repeats the whole kernel body back-to-back inside one NEFF
    (bench-only: isolates steady-state per-rep time from launch overhead).
    Semaphore thresholds are cumulative over the global tile index g."""
    nc = bass.Bass("TRN2", debug=False)
    A = nc.declare_dram_parameter("A", [ROWS], mybir.dt.float32, isOutput=False)
    B = nc.declare_dram_parameter("B", [ROWS, M], mybir.dt.float32, isOutput=False)
    C = nc.declare_dram_parameter("C", [ROWS, M], mybir.dt.float32, isOutput=True)

    # row r = p*T + t  (p outer, t inner) -> einops "(p t)"
    A2 = A.rearrange("(p t) -> p t", p=P)          # [128, 16]
    B3 = B.rearrange("(p t) m -> p t m", p=P)      # [128, 16, 4096]
    C3 = C.rearrange("(p t) m -> p t m", p=P)

    a_sb = nc.alloc_sbuf_tensor("a_sb", [P, T], mybir.dt.float32).ap()
    work = nc.alloc_sbuf_tensor("work", [P, K * M], mybir.dt.float32).ap()

    def slot(k):
        return work[:, k * M : (k + 1) * M]

    lda = nc.alloc_semaphore("lda")
    ld = [nc.alloc_semaphore(f"ld{k}") for k in range(K)]
    st = [nc.alloc_semaphore(f"st{k}") for k in range(K)]
    vs = nc.alloc_semaphore("vs")

    G = reps * T  # total tile count across reps; data tile = g % T

    with nc.Block() as block:

        @block.sync
        def _(sync: bass.BassEngine):
            sync.dma_start(out=a_sb, in_=A2).then_inc(lda, 16)
            for g in range(G):
                t, k = g % T, g % K
                if g >= K:
                    # slot free once store g-K fully landed
                    sync.wait_ge(st[k], 16 * (g // K))
                sync.dma_start(out=slot(k), in_=B3[:, t, :]).then_inc(ld[k], 16)

        @block.vector
        def _(vector: bass.BassEngine):
            vector.wait_ge(lda, 16)
            for g in range(G):
                t, k = g % T, g % K
                vector.wait_ge(ld[k], 16 * (g // K + 1))
                vector.tensor_scalar_mul(slot(k), slot(k), a_sb[:, t : t + 1]).then_inc(
                    vs, 1
                )

        @block.scalar
        def _(scalar: bass.BassEngine):
            for g in range(G):
                t, k = g % T, g % K
                scalar.wait_ge(vs, g + 1)
                scalar.dma_start(out=C3[:, t, :], in_=slot(k)).then_inc(st[k], 16)

    return nc


def kernel(A, B):
    global last_exec_time_ns
    A = np.ascontiguousarray(np.asarray(A), dtype=np.float32)
    B = np.ascontiguousarray(np.asarray(B), dtype=np.float32)
    assert A.shape == (N,) and B.shape == (N, M)

    if "nc" not in _nc_cache:
        _nc_cache["nc"] = _build_nc()
    nc = _nc_cache["nc"]

    in_maps = [
        {"A": A[c * ROWS : (c + 1) * ROWS], "B": B[c * ROWS : (c + 1) * ROWS]}
        for c in range(N_CORES)
    ]
    trace = bool(os.environ.get("BASS_KERNEL_TRACE"))
    res = run_bass_kernel_spmd(nc, in_maps, list(range(N_CORES)), trace=trace)
    last_exec_time_ns = res.exec_time_ns
    return np.concatenate([res.results[c]["C"] for c in range(N_CORES)], axis=0)


# revision 4
# speedup vs baseline: 16.1221x; 16.1221x over previous
"""Row-scale kernel: C = diag(A) @ B  (scale row i of B by A[i]).

Full shapes: A [16384] f32, B [16384, 4096] f32 -> C [16384, 4096] f32.
Sharding: pure data parallel over rows, 2048 rows per core on 8 cores.

Per-core layout: rows are interleaved over partitions, row r = p*T + t
(p = partition 0..127, t = tile 0..15).  That makes the per-tile scale
vector a_sb[:, t] a plain column of an A tile loaded with ONE contiguous
8 KiB DMA, and each B tile a clean 2D pattern (16 KiB contiguous per
partition, 256 KiB partition stride).

Raw Bass (no Tile framework) with an explicit software pipeline:
  SP sequencer   : B-tile loads  (HWDGE qSP ring)
  DVE            : per-partition scale multiply (in place)
  ACT sequencer  : C-tile stores (HWDGE qAct ring)
Per-buffer-slot semaphores; every instruction carries at most one
embedded wait (standalone sequencer waits otherwise) — the walrus
codegen rejects multi-wait TensorScalar instructions.
"""

import os

import numpy as np

import concourse.bass as bass
import concourse.mybir as mybir
from concourse.bass_utils import run_bass_kernel_spmd

N = 16384
M = 4096
N_CORES = 8
ROWS = N // N_CORES  # 2048 rows per core
P = 128              # SBUF partitions
T = ROWS // P        # 16 row-tiles per core
K = 8                # pipeline buffer slots (K * 16KiB = 128KiB per partition)

_nc_cache = {}
last_exec_time_ns = None


def _build_nc(reps=1, variant=0):
    """reps>1 repeats the whole kernel body back-to-back inside one NEFF
    (bench-only: isolates steady-state per-rep time from launch overhead);
    reps=0 builds an empty kernel (fixed-overhead measurement).
    Semaphore thresholds are cumulative over the global tile index g.

    variant 0: loads on SP ring, stores on ACT ring, 2 MiB tiles.
    variant 1: like 0 but paired tiles (4 MiB DMAs, two muls per slot).
    variant 2: loads split half/half across SP+ACT rings, stores on the
               gpsimd SWDGE queue.
    """
    nc = bass.Bass("TRN2", debug=False)
    A = nc.declare_dram_parameter("A", [ROWS], mybir.dt.float32, isOutput=False)
    B = nc.declare_dram_parameter("B", [ROWS, M], mybir.dt.float32, isOutput=False)
    C = nc.declare_dram_parameter("C", [ROWS, M], mybir.dt.float32, isOutput=True)

    if reps == 0:
        with nc.Block() as block:

            @block.sync
            def _(sync: bass.BassEngine):
                pass

        return nc

    # row r = p*T + t  (p outer, t inner) -> einops "(p t)"
    A2 = A.rearrange("(p t) -> p t", p=P)          # [128, 16]
    B3 = B.rearrange("(p t) m -> p t m", p=P)      # [128, 16, 4096]
    C3 = C.rearrange("(p t) m -> p t m", p=P)

    a_sb = nc.alloc_sbuf_tensor("a_sb", [P, T], mybir.dt.float32).ap()

    lda = nc.alloc_semaphore("lda")
    vs = nc.alloc_semaphore("vs")

    if variant in (0, 2):
        work = nc.alloc_sbuf_tensor("work", [P, K * M], mybir.dt.float32).ap()

        def slot(k):
            return work[:, k * M : (k + 1) * M]

        ld = [nc.alloc_semaphore(f"ld{k}") for k in range(K)]
        st = [nc.alloc_semaphore(f"st{k}") for k in range(K)]
        G = reps * T  # total tile count across reps; data tile = g % T

    if variant == 0:
        with nc.Block() as block:

            @block.sync
            def _(sync: bass.BassEngine):
                sync.dma_start(out=a_sb, in_=A2).then_inc(lda, 16)
                for g in range(G):
                    t, k = g % T, g % K
                    if g >= K:
                        # slot free once store g-K fully landed
                        sync.wait_ge(st[k], 16 * (g // K))
                    sync.dma_start(out=slot(k), in_=B3[:, t, :]).then_inc(ld[k], 16)

            @block.vector
            def _(vector: bass.BassEngine):
                vector.wait_ge(lda, 16)
                for g in range(G):
                    t, k = g % T, g % K
                    vector.wait_ge(ld[k], 16 * (g // K + 1))
                    vector.tensor_scalar_mul(
                        slot(k), slot(k), a_sb[:, t : t + 1]
                    ).then_inc(vs, 1)

            @block.scalar
            def _(scalar: bass.BassEngine):
                for g in range(G):
                    t, k = g % T, g % K
                    scalar.wait_ge(vs, g + 1)
                    scalar.dma_start(out=C3[:, t, :], in_=slot(k)).then_inc(st[k], 16)

    elif variant == 1:
        # paired tiles: one DMA covers data tiles (2j, 2j+1) -> 4 MiB
        KP = K // 2  # slots of 2*M floats
        TP = T // 2  # 8 paired tiles per rep
        work = nc.alloc_sbuf_tensor("work", [P, KP * 2 * M], mybir.dt.float32).ap()

        def pslot(k):
            return work[:, k * 2 * M : (k + 1) * 2 * M]

        ld = [nc.alloc_semaphore(f"ld{k}") for k in range(KP)]
        st = [nc.alloc_semaphore(f"st{k}") for k in range(KP)]
        G = reps * TP
        B4 = B.rearrange("(p j u) m -> p j (u m)", p=P, u=2)  # [128, 8, 8192]
        C4 = C.rearrange("(p j u) m -> p j (u m)", p=P, u=2)

        with nc.Block() as block:

            @block.sync
            def _(sync: bass.BassEngine):
                sync.dma_start(out=a_sb, in_=A2).then_inc(lda, 16)
                for g in range(G):
                    j, k = g % TP, g % KP
                    if g >= KP:
                        sync.wait_ge(st[k], 16 * (g // KP))
                    sync.dma_start(out=pslot(k), in_=B4[:, j, :]).then_inc(ld[k], 16)

            @block.vector
            def _(vector: bass.BassEngine):
                vector.wait_ge(lda, 16)
                for g in range(G):
                    j, k = g % TP, g % KP
                    vector.wait_ge(ld[k], 16 * (g // KP + 1))
                    s = pslot(k)
                    vector.tensor_scalar_mul(
                        s[:, :M], s[:, :M], a_sb[:, 2 * j : 2 * j + 1]
                    )
                    vector.tensor_scalar_mul(
                        s[:, M:], s[:, M:], a_sb[:, 2 * j + 1 : 2 * j + 2]
                    ).then_inc(vs, 1)

            @block.scalar
            def _(scalar: bass.BassEngine):
                for g in range(G):
                    j, k = g % TP, g % KP
                    scalar.wait_ge(vs, g + 1)
                    scalar.dma_start(out=C4[:, j, :], in_=pslot(k)).then_inc(st[k], 16)

    elif variant == 2:
        # loads: left half on SP ring, right half on ACT ring; stores SWDGE
        H = M // 2
        ldr = [nc.alloc_semaphore(f"ldr{k}") for k in range(K)]

        with nc.Block() as block:

            @block.sync
            def _(sync: bass.BassEngine):
                sync.dma_start(out=a_sb, in_=A2).then_inc(lda, 16)
                for g in range(G):
                    t, k = g % T, g % K
                    if g >= K:
                        sync.wait_ge(st[k], 16 * (g // K))
                    sync.dma_start(
                        out=slot(k)[:, :H], in_=B3[:, t, :H]
                    ).then_inc(ld[k], 16)

            @block.scalar
            def _(scalar: bass.BassEngine):
                for g in range(G):
                    t, k = g % T, g % K
                    if g >= K:
                        scalar.wait_ge(st[k], 16 * (g // K))
                    scalar.dma_start(
                        out=slot(k)[:, H:], in_=B3[:, t, H:]
                    ).then_inc(ldr[k], 16)

            @block.vector
            def _(vector: bass.BassEngine):
                vector.wait_ge(lda, 16)
                for g in range(G):
                    t, k = g % T, g % K
                    vector.wait_ge(ld[k], 16 * (g // K + 1))
                    vector.wait_ge(ldr[k], 16 * (g // K + 1))
                    vector.tensor_scalar_mul(
                        slot(k), slot(k), a_sb[:, t : t + 1]
                    ).then_inc(vs, 1)

            @block.gpsimd
            def _(gpsimd: bass.BassEngine):
                for g in range(G):
                    t, k = g % T, g % K
                    gpsimd.wait_ge(vs, g + 1)
                    gpsimd.dma_start(out=C3[:, t, :], in_=slot(k)).then_inc(st[k], 16)

    else:
        raise ValueError(variant)

    return nc


def kernel(A, B):
    global last_exec_time_ns
    A = np.ascontiguousarray(np.asarray(A), dtype=np.float32)
    B = np.ascontiguousarray(np.asarray(B), dtype=np.float32)
    assert A.shape == (N,) and B.shape == (N, M)

    if "nc" not in _nc_cache:
        _nc_cache["nc"] = _build_nc()
    nc = _nc_cache["nc"]

    in_maps = [
        {"A": A[c * ROWS : (c + 1) * ROWS], "B": B[c * ROWS : (c + 1) * ROWS]}
        for c in range(N_CORES)
    ]
    trace = bool(os.environ.get("BASS_KERNEL_TRACE"))
    res = run_bass_kernel_spmd(nc, in_maps, list(range(N_CORES)), trace=trace)
    last_exec_time_ns = res.exec_time_ns
    return np.concatenate([res.results[c]["C"] for c in range(N_CORES)], axis=0)
